# revision 3
# baseline (speedup 1.0000x reference)
"""AudioSNN Trainium2 kernel.

Two-layer leaky-integrate-and-fire SNN (snntorch Leaky, reset-by-subtract),
T=500 recurrent steps over batch 4096, data-parallel over 8 NeuronCores
(512 batch elements per core).

Math (per step t, reference):
    cur1 = x_t @ W1.T + b1
    m1   = beta*m1 + cur1 - H(m1_prev - 1)        # H(m1_prev-1) == spk1[t-1]
    spk1 = H(m1 - 1)
    cur2 = spk1 @ W2.T + b2
    m2   = beta*m2 + cur2 - spk2[t-1]
    spk2 = H(m2 - 1)    -> output [T, B, 5]

Device formulation (per core, full 512-batch tiles, all fp32):
  L1 state z1 = m1 - 1, spikes in sign form sgn = sign(z1) (spk = (sgn+1)/2):
    psum1 = W1aug^T.T @ x_aug     (bias b1+beta-1.5 via ones-row of x_aug)
    psum1 += (-0.5*I128) @ sgn[t-1]   (= -spk1[t-1] + const folded into bias)
    z1[t] = beta*z1[t-1] + psum1  (one DVE scalar_tensor_tensor)
    sgn[t] = Sign(z1[t])          (ACT engine)
  L2 state z2 = m2 - 1 - p, p = C2/(1-beta), C2 = 0.5*sum(W2,h) + b2 + beta-1:
    psum2 = (0.5*W2^T).T @ sgn[t] -> [5, 512]
    psum2 += (-I5) @ spk2[t-1]
    z2[t] = beta*z2[t-1] + psum2  (DVE)
    spk2[t] = (z2[t] > -p)        (DVE tensor_single_scalar, per-partition -p)
"""

import os
import sys

sys.path.insert(0, "/opt/trn_rl_repo")

from contextlib import ExitStack

import numpy as np

from concourse import bacc, mybir, tile
from concourse.bass_utils import run_bass_kernel_spmd

BETA = 0.9
T, F, H, O = 500, 40, 128, 5
NCORES = 8
BC = 512  # batch per core
CH = 20  # time steps per DMA chunk (must divide T)
F32 = mybir.dt.float32

MULT = mybir.AluOpType.mult
ADD = mybir.AluOpType.add
IS_GT = mybir.AluOpType.is_gt


def build(nc, n_steps=T, ch=CH, cut_pe_edges=False, variant="full"):
    """Emit the per-core program. x_aug layout: [n_chunks, (F+1)*ch*BC]."""
    n_chunks = n_steps // ch

    x_d = nc.dram_tensor(
        "x_aug", [n_chunks, (F + 1) * ch * BC], F32, kind="ExternalInput"
    ).ap()
    w1_d = nc.dram_tensor("w1aug", [F + 1, H], F32, kind="ExternalInput").ap()
    nhi_d = nc.dram_tensor("neg_half_i", [H, H], F32, kind="ExternalInput").ap()
    ni5_d = nc.dram_tensor("neg_i5", [O, O], F32, kind="ExternalInput").ap()
    w2h_d = nc.dram_tensor("w2half", [H, O], F32, kind="ExternalInput").ap()
    npp_d = nc.dram_tensor("negp", [O, 1], F32, kind="ExternalInput").ap()
    z2i_d = nc.dram_tensor("z2init", [O, BC], F32, kind="ExternalInput").ap()
    out_d = nc.dram_tensor("out", [O, n_steps * BC], F32, kind="ExternalOutput").ap()

    with tile.TileContext(nc) as tc, ExitStack() as ctx:
        const = ctx.enter_context(tc.tile_pool(name="const", bufs=1))
        state = ctx.enter_context(tc.tile_pool(name="state", bufs=1))
        xin = ctx.enter_context(tc.tile_pool(name="xin", bufs=2))
        outp = ctx.enter_context(tc.tile_pool(name="outp", bufs=2))
        ps1 = ctx.enter_context(tc.tile_pool(name="ps1", bufs=4, space="PSUM"))
        ps2 = ctx.enter_context(tc.tile_pool(name="ps2", bufs=4, space="PSUM"))

        w1_s = const.tile([F + 1, H], F32, tag="w1")
        nhi_s = const.tile([H, H], F32, tag="nhi")
        ni5_s = const.tile([O, O], F32, tag="ni5")
        w2h_s = const.tile([H, O], F32, tag="w2h")
        npp_s = const.tile([O, 1], F32, tag="npp")
        for s, d in [
            (w1_s, w1_d),
            (nhi_s, nhi_d),
            (ni5_s, ni5_d),
            (w2h_s, w2h_d),
            (npp_s, npp_d),
        ]:
            nc.sync.dma_start(out=s[:], in_=d[:])

        # Recurrent state, ping-pong buffered (index = t % 2).
        z1 = [state.tile([H, BC], F32, tag=f"z1_{pp}", name=f"z1_{pp}") for pp in range(2)]
        sg = [state.tile([H, BC], F32, tag=f"sg_{pp}", name=f"sg_{pp}") for pp in range(2)]
        z2 = [state.tile([O, BC], F32, tag=f"z2_{pp}", name=f"z2_{pp}") for pp in range(2)]
        spk0 = state.tile([O, BC], F32, tag="spk0")

        nc.vector.memset(z1[1][:], -1.0)  # m1(0)=0 -> z1=-1
        nc.vector.memset(sg[1][:], -1.0)  # sign(-1)
        nc.sync.dma_start(out=z2[1][:], in_=z2i_d[:])
        nc.vector.memset(spk0[:], 0.0)

        xt = None
        ot = None
        spk_prev = spk0[:]
        for t in range(n_steps):
            chk, st = divmod(t, ch)
            if st == 0:
                xt = xin.tile([F + 1, ch * BC], F32, tag="xt")
                nc.sync.dma_start(out=xt[:], in_=x_d[chk : chk + 1, :])
                ot = outp.tile([O, ch * BC], F32, tag="ot")
            cur, prv = t % 2, 1 - (t % 2)

            # ---- layer 1 ----
            p1 = ps1.tile([H, BC], F32, tag="p1")
            xs = xt[:, st * BC : (st + 1) * BC]
            nc.tensor.matmul(p1[:], w1_s[:], xs, start=True, stop=False)
            nc.tensor.matmul(
                p1[:], nhi_s[:], sg[1][:] if cut_pe_edges else sg[prv][:],
                start=False, stop=True,
            )
            if variant == "copystt":
                nc.vector.tensor_copy(z1[cur][:], p1[:])
            else:
                nc.vector.scalar_tensor_tensor(
                    z1[cur][:], z1[prv][:], BETA, p1[:], MULT, ADD
                )
            nc.scalar.sign(sg[cur][:], z1[cur][:])

            # ---- layer 2 ([5, 512]) ----
            if variant == "nol2":
                o_slice = ot[:, st * BC : (st + 1) * BC]
                nc.vector.tensor_single_scalar(o_slice, z1[cur][:O, :], npp_s[:], IS_GT)
                if st == ch - 1:
                    nc.sync.dma_start(
                        out=out_d[:, chk * ch * BC : (chk + 1) * ch * BC], in_=ot[:]
                    )
                continue
            p2 = ps2.tile([O, BC], F32, tag="p2")
            nc.tensor.matmul(
                p2[:], w2h_s[:], sg[1][:] if cut_pe_edges else sg[cur][:],
                start=True, stop=False,
            )
            nc.tensor.matmul(
                p2[:], ni5_s[:], spk0[:] if cut_pe_edges else spk_prev,
                start=False, stop=True,
            )
            if variant == "copystt":
                nc.vector.tensor_copy(z2[cur][:], p2[:])
            else:
                nc.vector.scalar_tensor_tensor(
                    z2[cur][:], z2[prv][:], BETA, p2[:], MULT, ADD
                )
            o_slice = ot[:, st * BC : (st + 1) * BC]
            nc.vector.tensor_single_scalar(o_slice, z2[cur][:], npp_s[:], IS_GT)
            spk_prev = o_slice

            if st == ch - 1:
                nc.sync.dma_start(
                    out=out_d[:, chk * ch * BC : (chk + 1) * ch * BC], in_=ot[:]
                )


def host_inputs(x, W1, b1, W2, b2, n_steps=T, ch=CH):
    """Shard + precompute all per-core device input arrays."""
    n_chunks = n_steps // ch
    x = np.asarray(x, np.float32)[:, :n_steps, :]
    W1 = np.asarray(W1, np.float32)
    b1 = np.asarray(b1, np.float32)
    W2 = np.asarray(W2, np.float32)
    b2 = np.asarray(b2, np.float32)

    # x: [B, T', F] -> per core [T', F, 512] -> augment ones -> chunked
    xs = x.reshape(NCORES, BC, n_steps, F).transpose(0, 2, 3, 1)  # [8,T',40,512]
    aug = np.empty((NCORES, n_steps, F + 1, BC), np.float32)
    aug[:, :, :F, :] = xs
    aug[:, :, F, :] = 1.0
    # [8, T', 41, 512] -> [8, n_chunks, 41, ch, 512] (chunk-major, partition dim 41)
    aug = aug.reshape(NCORES, n_chunks, ch, F + 1, BC).transpose(0, 1, 3, 2, 4)
    aug = np.ascontiguousarray(aug).reshape(NCORES, n_chunks, (F + 1) * ch * BC)

    w1aug = np.concatenate([W1.T, (b1 + BETA - 1.5)[None, :]], axis=0)  # [41,128]

    neg_half_i = (-0.5 * np.eye(H)).astype(np.float32)
    neg_i5 = (-np.eye(O)).astype(np.float32)
    w2half = (0.5 * W2.T).astype(np.float32)  # [128, 5]

    s2 = 0.5 * W2.sum(axis=1)  # [5]
    C2 = s2 + b2 + BETA - 1.0
    p = (C2 / (1.0 - BETA)).astype(np.float32)
    negp = (-p)[:, None].astype(np.float32)  # [5, 1]
    z2init = np.tile((-1.0 - p)[:, None], (1, BC)).astype(np.float32)  # [5, 512]

    shared = {
        "w1aug": np.ascontiguousarray(w1aug),
        "neg_half_i": neg_half_i,
        "neg_i5": neg_i5,
        "w2half": w2half,
        "negp": negp,
        "z2init": z2init,
    }
    return [{"x_aug": aug[c], **shared} for c in range(NCORES)]


def assemble(results, n_steps=T):
    """[O, T'*BC] f32 per core -> [T', B, O] float32."""
    outs = []
    for r in results:
        a = np.asarray(r["out"]).reshape(O, n_steps, BC).astype(np.float32)
        outs.append(a.transpose(1, 2, 0))  # [T', 512, 5]
    return np.concatenate(outs, axis=1)


LAST_RESULT = None  # BassKernelResults of the most recent run (for profiling)


def kernel(x, W1, b1, W2, b2):
    global LAST_RESULT
    in_maps = host_inputs(x, W1, b1, W2, b2)
    nc = bacc.Bacc("TRN2", target_bir_lowering=False, debug=False)
    build(nc)
    nc.compile()
    LAST_RESULT = run_bass_kernel_spmd(nc, in_maps, list(range(NCORES)))
    return assemble(LAST_RESULT.results)



# revision 5
# speedup vs baseline: 2.3809x; 2.3809x over previous
"""AudioSNN Trainium2 kernel (v2).

Two-layer leaky-integrate-and-fire SNN (snntorch Leaky, reset-by-subtract),
T=500 recurrent steps over batch 4096, data-parallel over 8 NeuronCores
(512 batch elements per core).

Math (per step t, reference):
    cur1 = x_t @ W1.T + b1
    m1   = beta*m1 + cur1 - H(m1_prev - 1)        # H(m1_prev-1) == spk1[t-1]
    spk1 = H(m1 - 1)
    cur2 = spk1 @ W2.T + b2
    m2   = beta*m2 + cur2 - spk2[t-1]
    spk2 = H(m2 - 1)    -> output [T, B, 5]

Device formulation (per core, all recurrent state fp32):
  Layer 1 state z1 = m1 - 1, spikes in sign form g = Sign(z1) in bf16
  (exact +-1), spk1 = (g+1)/2 with constants folded into the bias row.
  Batch 512 split into two independent halves A/B (256 each) so the two
  recurrence chains interleave across engines:
    p1_h  = W1aug^T.T @ x_aug_h     fp32 matmul, array rows 64h..64h+41
    p1_h += (-0.5*I128)_bf16 @ g_h[t-1]          (bf16, exact)
    z1_h[t] = beta*z1_h[t-1] + p1_h              (DVE scalar_tensor_tensor)
    g_h[t]  = Sign(z1_h[t])                      (ACT, bf16 out)
  Layer 2 in a column-grouped layout: batch quarter q (128 elems) maps to
  PSUM/SBUF partitions 32q+o (o<5), so elementwise work is [128,128] not
  [5,512]. z2 = m2 - 1 - p, p = C2/(1-beta), C2 = 0.5*sum(W2,h)+b2+beta-1:
    p2[32q:32q+5,:] = w2hi^T.T @ g[:,128q:...] + w2lo^T.T @ g[:,128q:...]
                      (W2 split hi+lo bf16, tile_position=(0,32q))
    p2 += (-Imask)_bf16 @ spk2'[t-1]             (diag 1 at used rows)
    z2'[t] = beta*z2'[t-1] + p2                  (DVE)
    spk2'[t] = (z2'[t] > -p')  -> bf16 0/1, DMA'd out per chunk
"""

import sys

sys.path.insert(0, "/opt/trn_rl_repo")

from contextlib import ExitStack

import ml_dtypes
import numpy as np

from concourse import bacc, mybir, tile
from concourse.bass_utils import run_bass_kernel_spmd

BETA = 0.9
T, F, H, O = 500, 40, 128, 5
NCORES = 8
BC = 512  # batch per core
G2 = 256  # batch per half (layer-1 chain granularity)
CH = 20  # time steps per DMA chunk (must divide T)
F32 = mybir.dt.float32
BF16 = mybir.dt.bfloat16

MULT = mybir.AluOpType.mult
ADD = mybir.AluOpType.add
IS_GT = mybir.AluOpType.is_gt


def build(nc, n_steps=T, ch=CH):
    n_chunks = n_steps // ch

    x_d = nc.dram_tensor(
        "x_aug", [n_chunks, 2 * (F + 1) * ch * G2], F32, kind="ExternalInput"
    ).ap()
    w1_d = nc.dram_tensor("w1aug", [F + 1, H], F32, kind="ExternalInput").ap()
    nhi_d = nc.dram_tensor("neg_half_i", [H, H], BF16, kind="ExternalInput").ap()
    nim_d = nc.dram_tensor("neg_i_mask", [H, H], BF16, kind="ExternalInput").ap()
    w2hi_d = nc.dram_tensor("w2hi", [H, O], BF16, kind="ExternalInput").ap()
    w2lo_d = nc.dram_tensor("w2lo", [H, O], BF16, kind="ExternalInput").ap()
    npp_d = nc.dram_tensor("negp", [H, 1], F32, kind="ExternalInput").ap()
    z2i_d = nc.dram_tensor("z2init", [H, H], F32, kind="ExternalInput").ap()
    out_d = nc.dram_tensor(
        "out", [n_chunks, 4 * O * ch * H], BF16, kind="ExternalOutput"
    ).ap()
    hs = F + 1  # 41 rows per half in x tile

    with tile.TileContext(nc) as tc, ExitStack() as ctx:
        const = ctx.enter_context(tc.tile_pool(name="const", bufs=1))
        state = ctx.enter_context(tc.tile_pool(name="state", bufs=1))
        xin = ctx.enter_context(tc.tile_pool(name="xin", bufs=2))
        outp = ctx.enter_context(tc.tile_pool(name="outp", bufs=2))
        ps1 = ctx.enter_context(tc.tile_pool(name="ps1", bufs=3, space="PSUM"))
        ps2 = ctx.enter_context(tc.tile_pool(name="ps2", bufs=2, space="PSUM"))

        w1_s = const.tile([64 + hs, H], F32, tag="w1")  # rows 0:41 and 64:105
        nhi_s = const.tile([H, H], BF16, tag="nhi")
        nim_s = const.tile([H, H], BF16, tag="nim")
        w2hi_s = const.tile([H, O], BF16, tag="w2hi")
        w2lo_s = const.tile([H, O], BF16, tag="w2lo")
        npp_s = const.tile([H, 1], F32, tag="npp")
        nc.sync.dma_start(out=w1_s[0:hs, :], in_=w1_d[:])
        nc.sync.dma_start(out=w1_s[64 : 64 + hs, :], in_=w1_d[:])
        for s, d in [
            (nhi_s, nhi_d),
            (nim_s, nim_d),
            (w2hi_s, w2hi_d),
            (w2lo_s, w2lo_d),
            (npp_s, npp_d),
        ]:
            nc.sync.dma_start(out=s[:], in_=d[:])

        # Recurrent state, ping-pong buffered (index = t % 2).
        z1 = [state.tile([H, BC], F32, tag=f"z1_{p}", name=f"z1_{p}") for p in range(2)]
        g = [state.tile([H, BC], BF16, tag=f"g_{p}", name=f"g_{p}") for p in range(2)]
        z2 = [state.tile([H, H], F32, tag=f"z2_{p}", name=f"z2_{p}") for p in range(2)]
        spk0 = state.tile([H, H], BF16, tag="spk0")

        nc.vector.memset(z1[1][:], -1.0)  # m1(0)=0 -> z1=-1
        nc.vector.memset(g[1][:], -1.0)  # sign(-1)
        nc.sync.dma_start(out=z2[1][:], in_=z2i_d[:])
        nc.vector.memset(spk0[:], 0.0)

        xt = None
        ot = None
        spk_prev = spk0[:]
        hlen = hs * ch * G2  # floats per half-chunk in x_aug
        for t in range(n_steps):
            chk, st = divmod(t, ch)
            if st == 0:
                xt = xin.tile([64 + hs, ch * G2], F32, tag="xt")
                nc.sync.dma_start(
                    out=xt[0:hs, :],
                    in_=x_d[chk : chk + 1, 0:hlen],
                )
                nc.sync.dma_start(
                    out=xt[64 : 64 + hs, :],
                    in_=x_d[chk : chk + 1, hlen : 2 * hlen],
                )
                ot = outp.tile([H, ch * H], BF16, tag="ot")
            cur, prv = t % 2, 1 - (t % 2)

            # ---- layer 1, two independent batch halves ----
            for h in range(2):
                bsl = slice(h * G2, (h + 1) * G2)
                p1 = ps1.tile([H, G2], F32, tag=f"p1_{h}")
                nc.tensor.matmul(
                    p1[:],
                    w1_s[64 * h : 64 * h + hs, :],
                    xt[64 * h : 64 * h + hs, st * G2 : (st + 1) * G2],
                    start=True,
                    stop=False,
                )
                nc.tensor.matmul(
                    p1[:], nhi_s[:], g[prv][:, bsl], start=False, stop=True
                )
                nc.vector.scalar_tensor_tensor(
                    z1[cur][:, bsl], z1[prv][:, bsl], BETA, p1[:], MULT, ADD
                )
                nc.scalar.sign(g[cur][:, bsl], z1[cur][:, bsl])

            # ---- layer 2, column-grouped [128, 128] ----
            p2 = ps2.tile([H, H], F32, tag="p2")
            for q in range(4):
                gq = g[cur][:, q * H : (q + 1) * H]
                nc.tensor.matmul(
                    p2[32 * q : 32 * q + O, :],
                    w2hi_s[:],
                    gq,
                    start=True,
                    stop=False,
                    tile_position=(0, 32 * q),
                )
                nc.tensor.matmul(
                    p2[32 * q : 32 * q + O, :],
                    w2lo_s[:],
                    gq,
                    start=False,
                    stop=False,
                    tile_position=(0, 32 * q),
                )
            nc.tensor.matmul(p2[:], nim_s[:], spk_prev, start=False, stop=True)
            nc.vector.scalar_tensor_tensor(
                z2[cur][:], z2[prv][:], BETA, p2[:], MULT, ADD
            )
            o_slice = ot[:, st * H : (st + 1) * H]
            nc.vector.tensor_single_scalar(o_slice, z2[cur][:], npp_s[:], IS_GT)
            spk_prev = o_slice

            if st == ch - 1:
                for q in range(4):
                    nc.sync.dma_start(
                        out=out_d[chk : chk + 1, q * O * ch * H : (q + 1) * O * ch * H],
                        in_=ot[32 * q : 32 * q + O, :],
                    )


def host_inputs(x, W1, b1, W2, b2, n_steps=T, ch=CH):
    """Shard + precompute all per-core device input arrays."""
    n_chunks = n_steps // ch
    x = np.asarray(x, np.float32)[:, :n_steps, :]
    W1 = np.asarray(W1, np.float32)
    b1 = np.asarray(b1, np.float32)
    W2 = np.asarray(W2, np.float32)
    b2 = np.asarray(b2, np.float32)

    # x: [B, T', F] -> per core [T', 41, 512] -> two batch halves, chunked
    xs = x.reshape(NCORES, BC, n_steps, F).transpose(0, 2, 3, 1)  # [8,T',40,512]
    aug = np.empty((NCORES, n_steps, F + 1, BC), np.float32)
    aug[:, :, :F, :] = xs
    aug[:, :, F, :] = 1.0
    # [8, T', 41, 2, 256] -> [8, n_chunks, 2, 41, ch, 256]
    aug = aug.reshape(NCORES, n_chunks, ch, F + 1, 2, G2).transpose(0, 1, 4, 3, 2, 5)
    aug = np.ascontiguousarray(aug).reshape(NCORES, n_chunks, 2 * (F + 1) * ch * G2)

    w1aug = np.concatenate([W1.T, (b1 + BETA - 1.5)[None, :]], axis=0)  # [41,128]

    neg_half_i = (-0.5 * np.eye(H)).astype(ml_dtypes.bfloat16)
    used = np.zeros(H, np.float32)
    for q in range(4):
        used[32 * q : 32 * q + O] = 1.0
    neg_i_mask = (-np.diag(used)).astype(ml_dtypes.bfloat16)

    w2half = (0.5 * W2.T).astype(np.float32)  # [128, 5]
    w2hi = w2half.astype(ml_dtypes.bfloat16)
    w2lo = (w2half - w2hi.astype(np.float32)).astype(ml_dtypes.bfloat16)

    s2 = 0.5 * W2.sum(axis=1)  # [5]
    C2 = s2 + b2 + BETA - 1.0
    p = (C2 / (1.0 - BETA)).astype(np.float32)
    negp = np.zeros((H, 1), np.float32)
    z2init = np.zeros((H, H), np.float32)
    for q in range(4):
        negp[32 * q : 32 * q + O, 0] = -p
        z2init[32 * q : 32 * q + O, :] = (-1.0 - p)[:, None]

    shared = {
        "w1aug": np.ascontiguousarray(w1aug),
        "neg_half_i": neg_half_i,
        "neg_i_mask": neg_i_mask,
        "w2hi": w2hi,
        "w2lo": w2lo,
        "negp": negp,
        "z2init": z2init,
    }
    return [{"x_aug": aug[c], **shared} for c in range(NCORES)]


def assemble(results, n_steps=T, ch=CH):
    """per-core out [n_chunks, 4*5*ch*128] bf16 -> [T', B, O] float32."""
    n_chunks = n_steps // ch
    outs = []
    for r in results:
        a = np.asarray(r["out"]).astype(np.float32)
        a = a.reshape(n_chunks, 4, O, ch, H)  # [k, q, o, st, b]
        # -> [t, b_in_core, o]
        a = a.transpose(0, 3, 1, 4, 2).reshape(n_steps, BC, O)
        outs.append(a)
    return np.concatenate(outs, axis=1)


LAST_RESULT = None  # BassKernelResults of the most recent run (for profiling)


def kernel(x, W1, b1, W2, b2):
    global LAST_RESULT
    in_maps = host_inputs(x, W1, b1, W2, b2)
    nc = bacc.Bacc("TRN2", target_bir_lowering=False, debug=False)
    build(nc)
    nc.compile()
    LAST_RESULT = run_bass_kernel_spmd(nc, in_maps, list(range(NCORES)))
    return assemble(LAST_RESULT.results)


# revision 7
# speedup vs baseline: 3.1986x; 1.3435x over previous
"""AudioSNN Trainium2 kernel (v3).

Two-layer leaky-integrate-and-fire SNN (snntorch Leaky, reset-by-subtract),
T=500 recurrent steps over batch 4096, data-parallel over 8 NeuronCores
(512 batch elements per core).

Math (per step t, reference):
    cur1 = x_t @ W1.T + b1
    m1   = beta*m1 + cur1 - H(m1_prev - 1)
    spk1 = H(m1 - 1)
    cur2 = spk1 @ W2.T + b2
    m2   = beta*m2 + cur2 - spk2[t-1]
    spk2 = H(m2 - 1)    -> output [T, B, 5]

Device formulation (per core, recurrent state fp32, matmuls fp16):
  cur1 is computed exactly (to spike precision; host-validated 0 flips) via
  a 3-term fp16 hi/lo split packed into TWO matmuls:
    x' = 16*x = xh + xl (fp16 pair), w' = W1.T/16 = wh + wl (fp16 pair)
    MM1: [wh; bias_hi]^T.T    @ [xh; 16s]      (K=41)
    MM2: [wl; bias_lo; wh]^T.T@ [xh; 16s; xl]  (K=81, cross terms stacked)
  Layer 1 state z1 = m1 - 1, spikes in sign form g = Sign(z1) (fp16, exact):
    p1 += (-0.5*I128)_fp16 @ g_h[t-1]  per batch half h (A: 0:256, B: 256:512)
    z1_h[t] = beta*z1_h[t-1] + p1_h    (DVE scalar_tensor_tensor)
    g_h[t]  = Sign(z1_h[t])            (ACT, fp16 out)
  Layer 2 column-grouped ([5,512] -> [128,128]: batch quarter q at
  partitions 32q+o) and scaled by S2=64 so the fp16 W2 hi/lo split stays
  normal-range. y2 = S2*(m2 - 1 - p), p = C2/(1-beta), C2 = 0.5*sum(W2,h)
  + b2 + beta - 1:
    p2[32q:32q+5,:] = w2hi^T.T @ g_q + w2lo^T.T @ g_q   (w2* = S2*0.5*W2.T)
    p2 += (-S2*Imask)_fp16 @ spk2'[t-1]
    y2[t] = beta*y2[t-1] + p2          (DVE)
    spk2'[t] = (y2[t] > -S2*p')        (DVE, fp16 0/1 out, DMA'd per chunk)
"""

import sys

sys.path.insert(0, "/opt/trn_rl_repo")

from contextlib import ExitStack

import numpy as np

from concourse import bacc, mybir, tile
from concourse.bass_utils import run_bass_kernel_spmd

BETA = 0.9
T, F, H, O = 500, 40, 128, 5
NCORES = 8
BC = 512  # batch per core
G2 = 256  # batch per half (layer-1 chain granularity)
CH = 20  # time steps per DMA chunk (must divide T)
S2 = 64.0  # layer-2 state scale
XS = 16.0  # x scale (W1 scaled by 1/XS)
F32 = mybir.dt.float32
FP16 = mybir.dt.float16

MULT = mybir.AluOpType.mult
ADD = mybir.AluOpType.add
IS_GT = mybir.AluOpType.is_gt

KA = F + 1  # MM1 contract: 40 xh rows + ones row
KB = 2 * F + 1  # MM2 contract: xh + ones + xl


def build(nc, n_steps=T, ch=CH):
    n_chunks = n_steps // ch

    x_d = nc.dram_tensor(
        "x_cat", [n_chunks, KB * ch * BC], FP16, kind="ExternalInput"
    ).ap()
    w1a_d = nc.dram_tensor("w1a", [KA, H], FP16, kind="ExternalInput").ap()
    w1b_d = nc.dram_tensor("w1b", [KB, H], FP16, kind="ExternalInput").ap()
    nhi_d = nc.dram_tensor("neg_half_i", [H, H], FP16, kind="ExternalInput").ap()
    nim_d = nc.dram_tensor("neg_i_mask", [H, H], FP16, kind="ExternalInput").ap()
    w2hi_d = nc.dram_tensor("w2hi", [H, O], FP16, kind="ExternalInput").ap()
    w2lo_d = nc.dram_tensor("w2lo", [H, O], FP16, kind="ExternalInput").ap()
    npp_d = nc.dram_tensor("negp", [H, 1], F32, kind="ExternalInput").ap()
    z2i_d = nc.dram_tensor("z2init", [H, H], F32, kind="ExternalInput").ap()
    out_d = nc.dram_tensor(
        "out", [n_chunks, 4 * O * ch * H], FP16, kind="ExternalOutput"
    ).ap()

    with tile.TileContext(nc) as tc, ExitStack() as ctx:
        const = ctx.enter_context(tc.tile_pool(name="const", bufs=1))
        state = ctx.enter_context(tc.tile_pool(name="state", bufs=1))
        xin = ctx.enter_context(tc.tile_pool(name="xin", bufs=2))
        outp = ctx.enter_context(tc.tile_pool(name="outp", bufs=2))
        ps1 = ctx.enter_context(tc.tile_pool(name="ps1", bufs=4, space="PSUM"))
        ps2 = ctx.enter_context(tc.tile_pool(name="ps2", bufs=3, space="PSUM"))

        w1a_s = const.tile([KA, H], FP16, tag="w1a")
        w1b_s = const.tile([KB, H], FP16, tag="w1b")
        nhi_s = const.tile([H, H], FP16, tag="nhi")
        nim_s = const.tile([H, H], FP16, tag="nim")
        w2hi_s = const.tile([H, O], FP16, tag="w2hi")
        w2lo_s = const.tile([H, O], FP16, tag="w2lo")
        npp_s = const.tile([H, 1], F32, tag="npp")
        for s, d in [
            (w1a_s, w1a_d),
            (w1b_s, w1b_d),
            (nhi_s, nhi_d),
            (nim_s, nim_d),
            (w2hi_s, w2hi_d),
            (w2lo_s, w2lo_d),
            (npp_s, npp_d),
        ]:
            nc.sync.dma_start(out=s[:], in_=d[:])

        # Recurrent state, ping-pong buffered (index = t % 2).
        z1 = [state.tile([H, BC], F32, tag=f"z1_{p}", name=f"z1_{p}") for p in range(2)]
        g = [state.tile([H, BC], FP16, tag=f"g_{p}", name=f"g_{p}") for p in range(2)]
        z2 = [state.tile([H, H], F32, tag=f"z2_{p}", name=f"z2_{p}") for p in range(2)]
        spk0 = state.tile([H, H], FP16, tag="spk0")

        nc.vector.memset(z1[1][:], -256.0)  # m1(0)=0 -> z1=-1, scale K1=256
        nc.vector.memset(g[1][:], -1.0)  # sign(-1)
        nc.sync.dma_start(out=z2[1][:], in_=z2i_d[:])
        nc.vector.memset(spk0[:], 0.0)

        xt = None
        ot = None
        spk_prev = spk0[:]
        for t in range(n_steps):
            chk, st = divmod(t, ch)
            if st == 0:
                xt = xin.tile([KB, ch * BC], FP16, tag="xt")
                nc.sync.dma_start(out=xt[:], in_=x_d[chk : chk + 1, :])
                ot = outp.tile([H, ch * H], FP16, tag="ot")
            cur, prv = t % 2, 1 - (t % 2)

            # ---- layer 1: cur1 (2 fp16 MMs) + per-half reset fold ----
            p1 = ps1.tile([H, BC], F32, tag="p1")
            xs = xt[:, st * BC : (st + 1) * BC]
            nc.tensor.matmul(p1[:], w1a_s[:], xs[0:KA, :], start=True, stop=False)
            nc.tensor.matmul(p1[:], w1b_s[:], xs[0:KB, :], start=False, stop=False)
            for h in range(2):
                bsl = slice(h * G2, (h + 1) * G2)
                nc.tensor.matmul(
                    p1[:, bsl], nhi_s[:], g[prv][:, bsl],
                    start=False, stop=(h == 1),
                )
            for h in range(2):
                bsl = slice(h * G2, (h + 1) * G2)
                nc.vector.scalar_tensor_tensor(
                    z1[cur][:, bsl], z1[prv][:, bsl], BETA, p1[:, bsl], MULT, ADD
                )
                nc.scalar.sign(g[cur][:, bsl], z1[cur][:, bsl])

            # ---- layer 2, column-grouped [128, 128], state scale S2 ----
            p2 = ps2.tile([H, H], F32, tag="p2")
            for q in range(4):
                gq = g[cur][:, q * H : (q + 1) * H]
                nc.tensor.matmul(
                    p2[32 * q : 32 * q + O, :],
                    w2hi_s[:],
                    gq,
                    start=True,
                    stop=False,
                    tile_position=(0, 32 * q),
                )
                nc.tensor.matmul(
                    p2[32 * q : 32 * q + O, :],
                    w2lo_s[:],
                    gq,
                    start=False,
                    stop=False,
                    tile_position=(0, 32 * q),
                )
            nc.tensor.matmul(p2[:], nim_s[:], spk_prev, start=False, stop=True)
            nc.vector.scalar_tensor_tensor(
                z2[cur][:], z2[prv][:], BETA, p2[:], MULT, ADD
            )
            o_slice = ot[:, st * H : (st + 1) * H]
            nc.vector.tensor_single_scalar(o_slice, z2[cur][:], npp_s[:], IS_GT)
            spk_prev = o_slice

            if st == ch - 1:
                for q in range(4):
                    nc.sync.dma_start(
                        out=out_d[chk : chk + 1, q * O * ch * H : (q + 1) * O * ch * H],
                        in_=ot[32 * q : 32 * q + O, :],
                    )


def _split16(a):
    hi = a.astype(np.float16)
    lo = (a - hi.astype(np.float32)).astype(np.float16)
    return hi, lo


def host_inputs(x, W1, b1, W2, b2, n_steps=T, ch=CH):
    """Shard + precompute all per-core device input arrays."""
    n_chunks = n_steps // ch
    x = np.asarray(x, np.float32)[:, :n_steps, :]
    W1 = np.asarray(W1, np.float32)
    b1 = np.asarray(b1, np.float32)
    W2 = np.asarray(W2, np.float32)
    b2 = np.asarray(b2, np.float32)

    # x*XS split into fp16 hi+lo; rows: [xh (40); 16s (1); xl (40)]
    xs = x.reshape(NCORES, BC, n_steps, F).transpose(0, 2, 3, 1)  # [8,T',40,512]
    xh = (XS * xs).astype(np.float16)
    xl = (XS * xs - xh.astype(np.float32)).astype(np.float16)
    cat = np.empty((NCORES, n_steps, KB, BC), np.float16)
    cat[:, :, 0:F, :] = xh
    cat[:, :, F, :] = np.float16(XS)
    cat[:, :, F + 1 :, :] = xl
    # [8, n_chunks, 81, ch, 512]
    cat = cat.reshape(NCORES, n_chunks, ch, KB, BC).transpose(0, 1, 3, 2, 4)
    x_cat = np.ascontiguousarray(cat).reshape(NCORES, n_chunks, KB * ch * BC)

    # W1.T*XS split (layer-1 state scaled by K1=XS*XS=256 so every fp16
    # hi/lo component stays in normal range); bias rows carry K1*b1'/XS
    # since the ones row in x carries XS
    w1t = W1.T * XS  # [40, 128]
    b1p = (b1 + BETA - 1.5) * XS
    w1hi, w1lo = _split16(w1t)
    bhi, blo = _split16(b1p[None, :])
    w1a = np.concatenate([w1hi, bhi], axis=0)  # [41,128]
    w1b = np.concatenate([w1lo, blo, w1hi], axis=0)  # [81,128]

    neg_half_i = (-0.5 * XS * XS * np.eye(H)).astype(np.float16)
    used = np.zeros(H, np.float32)
    for q in range(4):
        used[32 * q : 32 * q + O] = 1.0
    neg_i_mask = (-S2 * np.diag(used)).astype(np.float16)

    w2s = (S2 * 0.5 * W2.T).astype(np.float32)  # [128, 5]
    w2hi, w2lo = _split16(w2s)

    s2 = 0.5 * W2.sum(axis=1)  # [5]
    C2 = s2 + b2 + BETA - 1.0
    p = (C2 / (1.0 - BETA)).astype(np.float32)
    negp = np.zeros((H, 1), np.float32)
    z2init = np.zeros((H, H), np.float32)
    for q in range(4):
        negp[32 * q : 32 * q + O, 0] = -S2 * p
        z2init[32 * q : 32 * q + O, :] = (S2 * (-1.0 - p))[:, None]

    shared = {
        "w1a": w1a,
        "w1b": w1b,
        "neg_half_i": neg_half_i,
        "neg_i_mask": neg_i_mask,
        "w2hi": w2hi,
        "w2lo": w2lo,
        "negp": negp,
        "z2init": z2init,
    }
    return [{"x_cat": x_cat[c], **shared} for c in range(NCORES)]


def assemble(results, n_steps=T, ch=CH):
    """per-core out [n_chunks, 4*5*ch*128] fp16 -> [T', B, O] float32."""
    n_chunks = n_steps // ch
    outs = []
    for r in results:
        a = np.asarray(r["out"]).astype(np.float32)
        a = a.reshape(n_chunks, 4, O, ch, H)  # [k, q, o, st, b]
        a = a.transpose(0, 3, 1, 4, 2).reshape(n_steps, BC, O)
        outs.append(a)
    return np.concatenate(outs, axis=1)


LAST_RESULT = None  # BassKernelResults of the most recent run (for profiling)


def kernel(x, W1, b1, W2, b2):
    global LAST_RESULT
    in_maps = host_inputs(x, W1, b1, W2, b2)
    nc = bacc.Bacc("TRN2", target_bir_lowering=False, debug=False)
    build(nc)
    nc.compile()
    LAST_RESULT = run_bass_kernel_spmd(nc, in_maps, list(range(NCORES)))
    return assemble(LAST_RESULT.results)


# revision 10
# speedup vs baseline: 3.6582x; 1.1437x over previous
"""AudioSNN Trainium2 kernel (v4).

Two-layer leaky-integrate-and-fire SNN (snntorch Leaky, reset-by-subtract),
T=500 recurrent steps over batch 4096, data-parallel over 8 NeuronCores
(512 batch elements per core).

Math (per step t, reference):
    cur1 = x_t @ W1.T + b1
    m1   = beta*m1 + cur1 - H(m1_prev - 1)
    spk1 = H(m1 - 1)
    cur2 = spk1 @ W2.T + b2
    m2   = beta*m2 + cur2 - spk2[t-1]
    spk2 = H(m2 - 1)    -> output [T, B, 5]

Device formulation (per core; recurrent state fp32, matmuls fp16):
  cur1 is spike-exact (host-validated) via a 3-term fp16 hi/lo split packed
  into ONE K=122 matmul per batch half:
    x' = 16*x = xh + xl (fp16), w' = 16*W1.T = wh + wl (fp16)
    lhsT = [wh; bias_hi; wl; bias_lo; wh]  (122 x 128)
    rhs  = [xh; 16s;     xh; 16s;     xl]  -> p1 = K1*cur1', K1 = 256
  Layer-1 state z1 = K1*(m1 - 1).  Batch halves A (0:256) / B (256:512) run
  as independent recurrences in separate PSUM banks:
    p1_h += mask_h @ s_h[t-1]      (A: -0.5*K1*I @ sign; B: -K1*I @ spike)
    z1_h[t] = beta*z1_h[t-1] + p1_h        (DVE)
    s_A[t] = Sign(z1_A[t]) on ACT (+-1 fp16); s_B[t] = z1_B > 0 on DVE (0/1)
  Layer 2 column-grouped ([5,512] -> [128,128], batch quarter q at
  partitions 32q+o) and scaled by S2=64 (fp16-normal-range W2 split).
  y2 = S2*(m2 - 1 - p_q); quarters 0,1 consume sign-form (w2a = S2*0.5*W2.T,
  p from C2a = 0.5*sum(W2)+b2+beta-1), quarters 2,3 spike-form
  (w2b = S2*W2.T, C2b = b2+beta-1):
    p2[32q:32q+5,:] = w2{a,b}hi^T.T @ s_q + w2{a,b}lo^T.T @ s_q
    p2 += (-S2*Imask)_fp16 @ spk2'[t-1]
    y2[t] = beta*y2[t-1] + p2              (DVE)
    spk2'[t] = (y2[t] > -S2*p')            (DVE, fp16 0/1, DMA'd per chunk)
"""

import sys

sys.path.insert(0, "/opt/trn_rl_repo")

from contextlib import ExitStack

import numpy as np

from concourse import bacc, mybir, tile
from concourse.bass_utils import run_bass_kernel_spmd

BETA = 0.9
T, F, H, O = 500, 40, 128, 5
NCORES = 8
BC = 512  # batch per core
G2 = 256  # batch per half (layer-1 chain granularity)
CH = 20  # time steps per DMA chunk (must divide T)
S2 = 64.0  # layer-2 state scale
XS = 16.0  # x scale; W1 also scaled by XS; layer-1 state scale K1 = XS*XS
AHEAD = 2  # cur1 matmul emission lead (steps)
F32 = mybir.dt.float32
FP16 = mybir.dt.float16

MULT = mybir.AluOpType.mult
ADD = mybir.AluOpType.add
IS_GT = mybir.AluOpType.is_gt

KC = 3 * F + 2  # 122: [xh(40); ones; xh(40); ones; xl(40)]


def build(nc, n_steps=T, ch=CH):
    n_chunks = n_steps // ch

    x_d = nc.dram_tensor(
        "x_cat", [n_chunks, KC * ch * BC], FP16, kind="ExternalInput"
    ).ap()
    w1a_d = nc.dram_tensor("w1cat_a", [KC, H], FP16, kind="ExternalInput").ap()
    w1b_d = nc.dram_tensor("w1cat_b", [KC, H], FP16, kind="ExternalInput").ap()
    mska_d = nc.dram_tensor("mask_a", [H, H], FP16, kind="ExternalInput").ap()
    mskb_d = nc.dram_tensor("mask_b", [H, H], FP16, kind="ExternalInput").ap()
    nim_d = nc.dram_tensor("neg_i_mask", [H, H], FP16, kind="ExternalInput").ap()
    w2_d = [
        nc.dram_tensor(nm, [H, O], FP16, kind="ExternalInput").ap()
        for nm in ["w2ahi", "w2alo", "w2bhi", "w2blo"]
    ]
    npp_d = nc.dram_tensor("negp", [H, 1], F32, kind="ExternalInput").ap()
    z2i_d = nc.dram_tensor("z2init", [H, H], F32, kind="ExternalInput").ap()
    out_d = nc.dram_tensor(
        "out", [n_chunks, 4 * O * ch * H], FP16, kind="ExternalOutput"
    ).ap()

    with tile.TileContext(nc) as tc, ExitStack() as ctx:
        const = ctx.enter_context(tc.tile_pool(name="const", bufs=1))
        state = ctx.enter_context(tc.tile_pool(name="state", bufs=1))
        xin = ctx.enter_context(tc.tile_pool(name="xin", bufs=2))
        outp = ctx.enter_context(tc.tile_pool(name="outp", bufs=2))
        ps1 = ctx.enter_context(tc.tile_pool(name="ps1", bufs=3, space="PSUM"))
        ps2 = ctx.enter_context(tc.tile_pool(name="ps2", bufs=2, space="PSUM"))

        w1a_s = const.tile([KC, H], FP16, tag="w1a")
        w1b_s = const.tile([KC, H], FP16, tag="w1b")
        mska_s = const.tile([H, H], FP16, tag="mska")
        mskb_s = const.tile([H, H], FP16, tag="mskb")
        nim_s = const.tile([H, H], FP16, tag="nim")
        w2_s = [
            const.tile([H, O], FP16, tag=f"w2_{i}", name=f"w2_{i}")
            for i in range(4)
        ]
        npp_s = const.tile([H, 1], F32, tag="npp")
        for s, d in [
            (w1a_s, w1a_d),
            (w1b_s, w1b_d),
            (mska_s, mska_d),
            (mskb_s, mskb_d),
            (nim_s, nim_d),
            (npp_s, npp_d),
        ] + list(zip(w2_s, w2_d)):
            nc.sync.dma_start(out=s[:], in_=d[:])

        # Recurrent state, ping-pong buffered (index = t % 2).
        z1 = [state.tile([H, BC], F32, tag=f"z1_{p}", name=f"z1_{p}") for p in range(2)]
        g = [state.tile([H, BC], FP16, tag=f"g_{p}", name=f"g_{p}") for p in range(2)]
        z2 = [state.tile([H, H], F32, tag=f"z2_{p}", name=f"z2_{p}") for p in range(2)]
        spk0 = state.tile([H, H], FP16, tag="spk0")

        nc.vector.memset(z1[1][:], -XS * XS)  # m1(0)=0 -> z1 = -K1
        nc.vector.memset(g[1][:, 0:G2], -1.0)  # sign(-z)
        nc.vector.memset(g[1][:, G2:BC], 0.0)  # spike form
        nc.sync.dma_start(out=z2[1][:], in_=z2i_d[:])
        nc.vector.memset(spk0[:], 0.0)

        xts = {}
        p1s = {}
        ot = None
        spk_prev = spk0[:]

        def emit_c1(tf):
            """Emit the cur1 matmuls for step tf (one per batch half)."""
            chk, st = divmod(tf, ch)
            if st == 0:
                xt = xin.tile([KC, ch * BC], FP16, tag="xt")
                nc.sync.dma_start(out=xt[:], in_=x_d[chk : chk + 1, :])
                xts[chk] = xt
                xts.pop(chk - 2, None)
            xt = xts[chk]
            pair = []
            for hf in range(2):
                p1 = ps1.tile([H, G2], F32, tag=f"p1_{hf}")
                nc.tensor.matmul(
                    p1[:],
                    w1a_s[:] if hf == 0 else w1b_s[:],
                    xt[:, st * BC + hf * G2 : st * BC + (hf + 1) * G2],
                    start=True,
                    stop=False,
                )
                pair.append(p1)
            p1s[tf] = pair

        for tf in range(min(AHEAD, n_steps)):
            emit_c1(tf)

        for t in range(n_steps):
            chk, st = divmod(t, ch)
            if t + AHEAD < n_steps:
                emit_c1(t + AHEAD)
            if st == 0:
                ot = outp.tile([H, ch * H], FP16, tag="ot")
            cur, prv = t % 2, 1 - (t % 2)

            # ---- layer 1, independent batch halves ----
            for hf, msk in ((0, mska_s), (1, mskb_s)):
                bsl = slice(hf * G2, (hf + 1) * G2)
                p1 = p1s[t][hf]
                nc.tensor.matmul(
                    p1[:], msk[:], g[prv][:, bsl], start=False, stop=True
                )
                nc.vector.scalar_tensor_tensor(
                    z1[cur][:, bsl], z1[prv][:, bsl], BETA, p1[:], MULT, ADD
                )
                if hf == 0:
                    nc.scalar.sign(g[cur][:, bsl], z1[cur][:, bsl])
                else:
                    nc.vector.tensor_single_scalar(
                        g[cur][:, bsl], z1[cur][:, bsl], 0.0, IS_GT
                    )
            del p1s[t]

            # ---- layer 2, column-grouped [128, 128], state scale S2 ----
            p2 = ps2.tile([H, H], F32, tag="p2")
            for q in range(4):
                gq = g[cur][:, q * H : (q + 1) * H]
                hi, lo = (w2_s[0], w2_s[1]) if q < 2 else (w2_s[2], w2_s[3])
                nc.tensor.matmul(
                    p2[32 * q : 32 * q + O, :], hi, gq,
                    start=True, stop=False, tile_position=(0, 32 * q),
                )
                nc.tensor.matmul(
                    p2[32 * q : 32 * q + O, :], lo, gq,
                    start=False, stop=False, tile_position=(0, 32 * q),
                )
            nc.tensor.matmul(p2[:], nim_s[:], spk_prev, start=False, stop=True)
            nc.vector.scalar_tensor_tensor(
                z2[cur][:], z2[prv][:], BETA, p2[:], MULT, ADD
            )
            o_slice = ot[:, st * H : (st + 1) * H]
            nc.vector.tensor_single_scalar(o_slice, z2[cur][:], npp_s[:], IS_GT)
            spk_prev = o_slice

            if st == ch - 1:
                for q in range(4):
                    nc.sync.dma_start(
                        out=out_d[chk : chk + 1, q * O * ch * H : (q + 1) * O * ch * H],
                        in_=ot[32 * q : 32 * q + O, :],
                    )


def _split16(a):
    hi = a.astype(np.float16)
    lo = (a - hi.astype(np.float32)).astype(np.float16)
    return hi, lo


def host_inputs(x, W1, b1, W2, b2, n_steps=T, ch=CH):
    """Shard + precompute all per-core device input arrays."""
    n_chunks = n_steps // ch
    x = np.asarray(x, np.float32)[:, :n_steps, :]
    W1 = np.asarray(W1, np.float32)
    b1 = np.asarray(b1, np.float32)
    W2 = np.asarray(W2, np.float32)
    b2 = np.asarray(b2, np.float32)

    # x*XS split into fp16 hi+lo; rows: [xh(40); 16s; xh(40); 16s; xl(40)]
    xs = x.reshape(NCORES, BC, n_steps, F).transpose(0, 2, 3, 1)  # [8,T',40,512]
    xh = (XS * xs).astype(np.float16)
    xl = (XS * xs - xh.astype(np.float32)).astype(np.float16)
    cat = np.empty((NCORES, n_steps, KC, BC), np.float16)
    cat[:, :, 0:F, :] = xh
    cat[:, :, F, :] = np.float16(XS)
    cat[:, :, F + 1 : 2 * F + 1, :] = xh
    cat[:, :, 2 * F + 1, :] = np.float16(XS)
    cat[:, :, 2 * F + 2 :, :] = xl
    cat = cat.reshape(NCORES, n_chunks, ch, KC, BC).transpose(0, 1, 3, 2, 4)
    x_cat = np.ascontiguousarray(cat).reshape(NCORES, n_chunks, KC * ch * BC)

    # lhsT rows: [wh(40); bias_hi; wl(40); bias_lo; wh(40)]
    # half A (sign-form reset): bias b1 + beta - 1.5; half B (spike-form
    # reset): bias b1 + beta - 1.0
    w1t = W1.T * XS  # [40, 128]
    w1hi, w1lo = _split16(w1t)

    def w1cat_for(bias_shift):
        b1p = (b1 + BETA - bias_shift) * XS
        bhi, blo = _split16(b1p[None, :])
        return np.concatenate([w1hi, bhi, w1lo, blo, w1hi], axis=0)  # [122,128]

    w1cat_a = w1cat_for(1.5)
    w1cat_b = w1cat_for(1.0)

    K1 = XS * XS
    mask_a = (-0.5 * K1 * np.eye(H)).astype(np.float16)  # sign-form reset
    mask_b = (-K1 * np.eye(H)).astype(np.float16)  # spike-form reset
    used = np.zeros(H, np.float32)
    for q in range(4):
        used[32 * q : 32 * q + O] = 1.0
    neg_i_mask = (-S2 * np.diag(used)).astype(np.float16)

    w2a = (S2 * 0.5 * W2.T).astype(np.float32)  # sign-form quarters
    w2b = (S2 * W2.T).astype(np.float32)  # spike-form quarters
    w2ahi, w2alo = _split16(w2a)
    w2bhi, w2blo = _split16(w2b)

    C2a = 0.5 * W2.sum(axis=1) + b2 + BETA - 1.0
    C2b = b2 + BETA - 1.0
    pa = (C2a / (1.0 - BETA)).astype(np.float32)
    pb = (C2b / (1.0 - BETA)).astype(np.float32)
    negp = np.zeros((H, 1), np.float32)
    z2init = np.zeros((H, H), np.float32)
    for q in range(4):
        p = pa if q < 2 else pb
        negp[32 * q : 32 * q + O, 0] = -S2 * p
        z2init[32 * q : 32 * q + O, :] = (S2 * (-1.0 - p))[:, None]

    shared = {
        "w1cat_a": w1cat_a,
        "w1cat_b": w1cat_b,
        "mask_a": mask_a,
        "mask_b": mask_b,
        "neg_i_mask": neg_i_mask,
        "w2ahi": w2ahi,
        "w2alo": w2alo,
        "w2bhi": w2bhi,
        "w2blo": w2blo,
        "negp": negp,
        "z2init": z2init,
    }
    return [{"x_cat": x_cat[c], **shared} for c in range(NCORES)]


def assemble(results, n_steps=T, ch=CH):
    """per-core out [n_chunks, 4*5*ch*128] fp16 -> [T', B, O] float32."""
    n_chunks = n_steps // ch
    outs = []
    for r in results:
        a = np.asarray(r["out"]).astype(np.float32)
        a = a.reshape(n_chunks, 4, O, ch, H)  # [k, q, o, st, b]
        a = a.transpose(0, 3, 1, 4, 2).reshape(n_steps, BC, O)
        outs.append(a)
    return np.concatenate(outs, axis=1)


LAST_RESULT = None  # BassKernelResults of the most recent run (for profiling)


def kernel(x, W1, b1, W2, b2):
    global LAST_RESULT
    in_maps = host_inputs(x, W1, b1, W2, b2)
    nc = bacc.Bacc("TRN2", target_bir_lowering=False, debug=False)
    build(nc)
    nc.compile()
    LAST_RESULT = run_bass_kernel_spmd(nc, in_maps, list(range(NCORES)))
    return assemble(LAST_RESULT.results)
